# revision 5
# baseline (speedup 1.0000x reference)
"""AAUpsample1d Trainium2 kernel (fp8 DoubleRow, no-halo edition).

Reference computation (per batch element):
  y   = conv_transpose1d(x, conv_w, stride=2, k=3) + conv_b        # [512, 8192]
  y   = depthwise_conv1d(y, aa_kernel, k=17, same)                 # anti-alias
  out = proj_w @ y + proj_b                                        # 1x1 projection

Algebraic restructuring:
  * depthwise AA commutes with the 1x1 projection; fold proj into the three
    polyphase matrices M_k = proj_w @ conv_w[:,:,k]:
        z[2u]   = M1 @ x[u]
        z[2u+1] = M0 @ x[u] + M2 @ x[u+1]
        out     = AA(z) + (sum(aa) * proj_w @ conv_b + proj_b)
  * z is produced in 32 NON-overlapping 128-u tiles (z idx block = 256 per
    tile).  AA(z) runs as banded-Toeplitz matmuls; each PSUM bank a holds out
    cols [512a-8, 512a+504) so a bank needs only tiles 2a-1 (16 cols, matrix
    R_C), 2a (272 cols, R_A) and 2a+1 (256 cols, R_A[:, :, :256]) — never a
    FUTURE tile, so banks close as soon as their odd tile's z exists.  The
    final 8 out cols form a tiny "mini pair" fed by tile 31 via R_C[:, :, :8].

fp8 acceleration (same numerics as the halo baseline):
  * All heavy matmuls use MatmulPerfMode.DoubleRow (fp8, two K=128 slabs per
    pass, 0.5 PE cycles per output column).
  * Accuracy via hi/lo splitting: A ~ e4m3(A) + e5m2(A - e4m3(A));
    z = xh@Mh + xl@Mh + xh@Ml (xl@Ml is O(delta^2), dropped).  M is pre-scaled
    by SM=1024; undone in the PSUM->SBUF z copies.
  * AA Toeplitz in e4m3 at host-optimized global scale SR (undone in output
    copies); z enters AA as zhi(e4m3)+zlo(e5m2), phase pairs (ze,zo) in the
    two DoubleRow K slots.
  * ze/zo accumulate in one merged 2-bank PSUM tile so the z->SBUF quantized
    copies are single 1024-element instructions (ACT for zhi, DVE for zlo).
  * Tail tile 31 takes a bf16 fast path (parallel single copies, bf16 AA) so
    the final AA is not gated by the serial hi->lo copy chain.
  * Output written bf16, widened to f32 on host.

Sharding: pure data-parallel, one batch element per NeuronCore (B=8 = n_cores).
"""

import numpy as np
import ml_dtypes

import concourse.bass as bass
import concourse.mybir as mybir
import concourse.tile as tile
from concourse import bacc
from concourse.bass_utils import run_bass_kernel_spmd

BF16 = ml_dtypes.bfloat16
E4 = ml_dtypes.float8_e4m3
E5 = ml_dtypes.float8_e5m2

B, D, L = 8, 512, 4096
LOUT = 2 * L
KSIZE = 17
NJ = 32                           # non-overlapping z tiles of 128 u each
NP = 16                           # full PSUM pairs (bank a = cols 512a-8..512a+504)
ACOLS = 272                       # R_A columns (A' pass); B' uses first 256
CCOLS = 16                        # R_C columns (C' pass); mini uses first 8
XCOLS = L + 16                    # x cols + right pad (only x[4096] is read)
NCC = D // 128                    # 4 channel chunks
NQ = 2                            # DoubleRow ic-chunk pairs per 512 contraction
SM = 1024.0                       # e4m3 scale for the M matrices
# x segments: small first segment so tile 0 unblocks early; the rest stream in
SEGSTARTS = [0, 2, 5, 8, 11, 14, 17, 20, 23, 26, 29]
NSEG = len(SEGSTARTS)
_SEGENDS = SEGSTARTS[1:] + [NJ]

_CACHE = {}


def _build_bass(srinv):
    nc = bacc.Bacc("TRN2", target_bir_lowering=False)
    f32 = mybir.dt.float32
    bf16 = mybir.dt.bfloat16
    e4 = mybir.dt.float8e4
    e5 = mybir.dt.float8e5
    DR = mybir.MatmulPerfMode.DoubleRow
    Ident = mybir.ActivationFunctionType.Identity

    # x pre-windowed: [p, J, q, tap, slot, m]; value = x[(2q+s)*128+p, 128J+t+m]
    xh_d = nc.dram_tensor("xh", [128, NJ, NQ, 2, 2, 128], e4, kind="ExternalInput")
    xl_d = nc.dram_tensor("xl", [128, NJ, NQ, 2, 2, 128], e5, kind="ExternalInput")
    mth_d = nc.dram_tensor("mth", [D, 3, D], e4, kind="ExternalInput")
    mtl_d = nc.dram_tensor("mtl", [D, 3, D], e5, kind="ExternalInput")
    ra_d = nc.dram_tensor("ra", [128, 2, ACOLS], e4, kind="ExternalInput")
    rab_d = nc.dram_tensor("rab", [128, 2, ACOLS], bf16, kind="ExternalInput")
    rc_d = nc.dram_tensor("rc", [128, 2, CCOLS], e4, kind="ExternalInput")
    rcb_d = nc.dram_tensor("rcb", [128, 2, CCOLS], bf16, kind="ExternalInput")
    bias_d = nc.dram_tensor("bias", [D], f32, kind="ExternalInput")
    out_d = nc.dram_tensor("out", [D, LOUT], bf16, kind="ExternalOutput")

    with tile.TileContext(nc) as tc:
        with (
            tc.tile_pool(name="const", bufs=1) as cpool,
            tc.tile_pool(name="zhi", bufs=3) as zhpool,
            tc.tile_pool(name="zlo", bufs=3) as zlpool,
            tc.tile_pool(name="osb", bufs=3) as opool,
            tc.tile_pool(name="zmm", bufs=2, space="PSUM") as zmm,
            tc.tile_pool(name="aa", bufs=4, space="PSUM") as aamm,
        ):
            # ---- constants / inputs ----
            # DMA emission order = execution order on the shared DMA engines:
            # the first tile's operands first (M1 tap + first x segment) so the
            # first real matmul unblocks as early as possible.
            mtsh = [cpool.tile([128, 2, 3, D], e4, name=f"mtsh{q}")
                    for q in range(NQ)]
            mtsl = [cpool.tile([128, 2, 3, D], e5, name=f"mtsl{q}")
                    for q in range(NQ)]
            seglen = [_SEGENDS[s] - SEGSTARTS[s] for s in range(NSEG)]
            xhsegs = [cpool.tile([128, seglen[s], NQ, 2, 2, 128], e4,
                                 name=f"xhseg{s}") for s in range(NSEG)]
            xlsegs = [cpool.tile([128, seglen[s], NQ, 2, 2, 128], e5,
                                 name=f"xlseg{s}") for s in range(NSEG)]

            def _dma_mts_tap(dst, src_d, q, k):
                src = src_d[256 * q:256 * (q + 1), k:k + 1]
                nc.sync.dma_start(
                    dst[q][:, :, k:k + 1],
                    src.rearrange("(s p) k o -> p s k o", p=128))

            # tap order (1, 0, 2) matches first-use order inside emit_main
            _dma_mts_tap(mtsh, mth_d, 0, 1)
            nc.sync.dma_start(xhsegs[0][:], xh_d[:, SEGSTARTS[0]:_SEGENDS[0]])
            _dma_mts_tap(mtsh, mth_d, 0, 0)
            _dma_mts_tap(mtsh, mth_d, 0, 2)
            for k in (1, 0, 2):
                _dma_mts_tap(mtsh, mth_d, 1, k)
            for q in range(NQ):
                for k in (1, 0, 2):
                    _dma_mts_tap(mtsl, mtl_d, q, k)
            nc.sync.dma_start(xlsegs[0][:], xl_d[:, SEGSTARTS[0]:_SEGENDS[0]])

            ras = cpool.tile([128, 2, ACOLS], e4, name="ras")
            nc.sync.dma_start(ras[:], ra_d[:])
            rcs = cpool.tile([128, 2, CCOLS], e4, name="rcs")
            nc.sync.dma_start(rcs[:], rc_d[:])
            rasb = cpool.tile([128, 2, ACOLS], bf16, name="rasb")
            nc.sync.dma_start(rasb[:], rab_d[:])
            rcsb = cpool.tile([128, 2, CCOLS], bf16, name="rcsb")
            nc.sync.dma_start(rcsb[:], rcb_d[:])
            biast = cpool.tile([128, NCC], f32, name="biast")
            nc.sync.dma_start(biast[:], bias_d.rearrange("(o p) -> p o", p=128))

            # all remaining x segments up front: a queued output DMA waiting on
            # its copies would head-of-line-block later x segments in the DGE
            # queue, starving the compute wavefront mid-kernel
            for s in range(1, NSEG):
                nc.sync.dma_start(xhsegs[s][:], xh_d[:, SEGSTARTS[s]:_SEGENDS[s]])
                nc.sync.dma_start(xlsegs[s][:], xl_d[:, SEGSTARTS[s]:_SEGENDS[s]])

            zs = [None] * NJ
            pair_psum = {}
            out_r = out_d.rearrange("(cc p) l -> p cc l", p=128)

            # warm-up matmuls on a zeroed scratch tile: ramps the PE clock
            # (HAM / p-state) out of its cold state while the first DMAs land
            NWARM = 56
            wsb = cpool.tile([128, 64], bf16, name="wsb")
            nc.vector.memset(wsb[:], 0.0)
            wps = zmm.tile([128, 2, D], f32, tag="zmm", name="wps")
            for _ in range(NWARM):
                nc.tensor.matmul(wps[:64, 0, :64], lhsT=wsb[:], rhs=wsb[:],
                                 start=True, stop=True)

            seg_of = {J: s for s in range(NSEG)
                      for J in range(SEGSTARTS[s], _SEGENDS[s])}

            def emit_main(J):
                # z tile J covers u in [128J, 128J+128)
                s = seg_of[J]
                Js = J - SEGSTARTS[s]
                xh_t, xl_t = xhsegs[s], xlsegs[s]
                # 18 DoubleRow matmuls into one merged 2-bank PSUM tile
                # (ze = zp[:, 0], zo = zp[:, 1]); products grouped to match
                # DMA arrival order (mtsh, xh, mtl, xl).
                zp = zmm.tile([128, 2, D], f32, tag="zmm", name="zp")
                ne = no = 0
                for xt, mts in ((xh_t, mtsh), (xh_t, mtsl), (xl_t, mtsh)):
                    for q in range(NQ):
                        nc.tensor.matmul(
                            zp[:, 0], lhsT=xt[:, Js, q, 0],
                            rhs=mts[q][:, :, 1], perf_mode=DR,
                            start=(ne == 0), stop=(ne == 3 * NQ - 1),
                        )
                        ne += 1
                        for k, tap in ((0, 0), (2, 1)):
                            nc.tensor.matmul(
                                zp[:, 1], lhsT=xt[:, Js, q, tap],
                                rhs=mts[q][:, :, k], perf_mode=DR,
                                start=(no == 0), stop=(no == 6 * NQ - 1),
                            )
                            no += 1
                # PSUM -> SBUF quantized copies, into the AA-ready layout
                # [p, cc, phase, m] (dense 256-run per cc for DoubleRow lhsT).
                zv = zp[:].rearrange("p h (c m) -> p c h m", c=NCC)
                if J == NJ - 1:
                    # tail fast-path: single bf16 copies on two engines in
                    # parallel (the hi->lo chain would serialize ~2.2us)
                    zbf = zhpool.tile([128, NCC, 2, 128], bf16, tag="zhi",
                                      name="zbf")
                    nc.scalar.activation(zbf[:, :, 0],
                                         zp[:, 0].rearrange("p (c m) -> p c m",
                                                            c=NCC),
                                         Ident, scale=1.0 / SM)
                    nc.vector.tensor_scalar_mul(
                        zbf[:, :, 1],
                        zp[:, 1].rearrange("p (c m) -> p c m", c=NCC),
                        1.0 / SM)
                    zs[J] = (zbf, None)
                    return
                # zhi = e4m3(psum/SM) on ACT; zlo = e5m2(psum/SM - zhi) on DVE
                zhi = zhpool.tile([128, NCC, 2, 128], e4, tag="zhi", name="zhi")
                zlo = zlpool.tile([128, NCC, 2, 128], e5, tag="zlo", name="zlo")
                nc.scalar.activation(zhi[:], zv, Ident, scale=1.0 / SM)
                for h in range(2):
                    nc.vector.scalar_tensor_tensor(
                        out=zlo[:, :, h],
                        in0=zp[:, h].rearrange("p (c m) -> p c m", c=NCC),
                        scalar=1.0 / SM,
                        in1=zhi[:, :, h], op0=mybir.AluOpType.mult,
                        op1=mybir.AluOpType.subtract,
                    )
                zs[J] = (zhi, zlo)

            def emit_aa_of(t):
                a, half = divmod(t, 2)
                if half == 0:
                    # open pair a: C' (z[2a-1] -> cols [0,16)) then
                    # A' (z[2a] -> cols [0,272))
                    pair_psum[a] = [
                        aamm.tile([128, 512], f32, tag="aa", name=f"aa_ps{cc}")
                        for cc in range(NCC)
                    ]
                    zc = zs[2 * a - 1] if a > 0 else None
                    zhi, zlo = zs[t]
                    for cc in range(NCC):
                        ps = pair_psum[a][cc]
                        first = True
                        if zc is not None:
                            nc.tensor.matmul(
                                ps[:, 0:CCOLS], lhsT=zc[0][:, cc], rhs=rcs[:],
                                perf_mode=DR, start=True, stop=False)
                            nc.tensor.matmul(
                                ps[:, 0:CCOLS], lhsT=zc[1][:, cc], rhs=rcs[:],
                                perf_mode=DR, start=False, stop=False)
                            first = False
                        nc.tensor.matmul(
                            ps[:, 0:ACOLS], lhsT=zhi[:, cc], rhs=ras[:],
                            perf_mode=DR, start=first, stop=False)
                        nc.tensor.matmul(
                            ps[:, 0:ACOLS], lhsT=zlo[:, cc], rhs=ras[:],
                            perf_mode=DR, start=False, stop=False)
                    return
                # half == 1: B' (z[2a+1] -> cols [256,512)), close pair a,
                # copies + DMA
                zhi, zlo = zs[t]
                for cc in range(NCC):
                    dst = pair_psum[a][cc][:, 256:512]
                    if zlo is None:          # bf16 tail path
                        nc.tensor.matmul(
                            dst, lhsT=zhi[:, cc, 0], rhs=rasb[:, 0, :256],
                            start=False, stop=False)
                        nc.tensor.matmul(
                            dst, lhsT=zhi[:, cc, 1], rhs=rasb[:, 1, :256],
                            start=False, stop=True)
                        continue
                    nc.tensor.matmul(
                        dst, lhsT=zhi[:, cc], rhs=ras[:, :, :256],
                        perf_mode=DR, start=False, stop=False)
                    nc.tensor.matmul(
                        dst, lhsT=zlo[:, cc], rhs=ras[:, :, :256],
                        perf_mode=DR, start=False, stop=True)
                # out copies: bank a = out cols [512a-8, 512a+504); pair 0
                # skips its first 8 (garbage) psum cols.  GPSIMD can't read
                # PSUM, so split 3:1 ACT:DVE (DVE also carries the zlo chain).
                skip = 8 if a == 0 else 0
                ncols = 512 - skip
                lbase = max(0, 512 * a - 8)
                osb = opool.tile([128, NCC, 512], bf16, tag="osb", name="osb")
                for cc in range(NCC):
                    if cc != 3:
                        nc.scalar.activation(
                            osb[:, cc, :ncols],
                            pair_psum[a][cc][:, skip:512],
                            Ident, bias=biast[:, cc:cc + 1], scale=srinv)
                    else:
                        nc.vector.scalar_tensor_tensor(
                            out=osb[:, cc, :ncols],
                            in0=pair_psum[a][cc][:, skip:512],
                            scalar=srinv,
                            in1=biast[:, cc:cc + 1].to_broadcast((128, ncols)),
                            op0=mybir.AluOpType.mult, op1=mybir.AluOpType.add)
                del pair_psum[a]
                if a == NP - 1:
                    # last pair: per-cc DMAs so the final bytes leave ASAP
                    for cc in range(NCC):
                        nc.sync.dma_start(
                            out_r[:, cc:cc + 1, lbase:lbase + ncols],
                            osb[:, cc:cc + 1, :ncols])
                else:
                    # two half-DMAs: shorter bursts interleave more fairly
                    # with the x-segment DMAs on the shared DMA engines
                    nc.sync.dma_start(
                        out_r[:, 0:2, lbase:lbase + ncols], osb[:, 0:2, :ncols])
                    nc.sync.dma_start(
                        out_r[:, 2:4, lbase:lbase + ncols], osb[:, 2:4, :ncols])

            def emit_mini():
                # final 8 out cols [8184, 8192) from tile 31 via R_C[:, :, :8];
                # psum borrowed from the zmm pool (free; aa pool would stall on
                # pair 15's copies)
                zbf, _ = zs[NJ - 1]
                mps = zmm.tile([128, 2, D], f32, tag="zmm", name="mps")
                for cc in range(NCC):
                    nc.tensor.matmul(
                        mps[:, 0, 8 * cc:8 * cc + 8], lhsT=zbf[:, cc, 0],
                        rhs=rcsb[:, 0, :8], start=True, stop=False)
                    nc.tensor.matmul(
                        mps[:, 0, 8 * cc:8 * cc + 8], lhsT=zbf[:, cc, 1],
                        rhs=rcsb[:, 1, :8], start=False, stop=True)
                osbm = opool.tile([128, NCC, 512], bf16, tag="osb", name="osbm")
                for cc in range(NCC):
                    if cc < 2:
                        nc.scalar.activation(
                            osbm[:, cc, :8], mps[:, 0, 8 * cc:8 * cc + 8],
                            Ident, bias=biast[:, cc:cc + 1], scale=srinv)
                    else:
                        nc.vector.scalar_tensor_tensor(
                            out=osbm[:, cc, :8],
                            in0=mps[:, 0, 8 * cc:8 * cc + 8], scalar=srinv,
                            in1=biast[:, cc:cc + 1].to_broadcast((128, 8)),
                            op0=mybir.AluOpType.mult, op1=mybir.AluOpType.add)
                nc.sync.dma_start(out_r[:, :, LOUT - 8:LOUT], osbm[:, :, :8])

            # software-pipelined emission: AA(J-1) after main(J) so the PE
            # never waits on the z copies.
            for J in range(NJ):
                emit_main(J)
                if J >= 1:
                    emit_aa_of(J - 1)
            emit_aa_of(NJ - 1)
            emit_mini()

    nc.compile()
    return nc


def _host_weights(conv_w, conv_b, aa_kernel, proj_w, proj_b):
    aa = np.asarray(aa_kernel, np.float32)
    proj_w = np.asarray(proj_w, np.float32)
    # fold the projection into the three polyphase matrices, hi/lo split
    m = [proj_w @ np.asarray(conv_w, np.float32)[:, :, k] for k in range(3)]
    mh = [(mk * SM).astype(E4) for mk in m]
    ml = [(mk * SM - mhk.astype(np.float32)).astype(E5)
          for mk, mhk in zip(m, mh)]
    mth_np = np.ascontiguousarray(np.stack([mk.T for mk in mh], axis=1))
    mtl_np = np.ascontiguousarray(np.stack([mk.T for mk in ml], axis=1))

    # global scale for the e4m3 AA taps, optimized against the actual kernel
    nz = aa[np.abs(aa) > 1e-9]
    best = (np.inf, 1.0)
    for s in np.geomspace(0.5, 4.0, 8001):
        q = (nz * s).astype(E4).astype(np.float32) / s
        err = float(np.sum((q - nz) ** 2))
        if err < best[0]:
            best = (err, float(s))
    sr = best[1]

    u = np.arange(128)[:, None, None]
    ph = np.arange(2)[None, :, None]
    cA = np.arange(ACOLS)[None, None, :]
    tA = 2 * u + ph + 16 - cA
    r_a = np.where((tA >= 0) & (tA < KSIZE),
                   sr * aa[np.clip(tA, 0, KSIZE - 1)], 0.0)
    cC = np.arange(CCOLS)[None, None, :]
    tC = 2 * u + ph - 240 - cC
    r_c = np.where((tC >= 0) & (tC < KSIZE),
                   sr * aa[np.clip(tC, 0, KSIZE - 1)], 0.0)
    ra_np = r_a.astype(E4)
    rab_np = r_a.astype(BF16)     # bf16 twin keeps the sr scale (shared psum)
    rc_np = r_c.astype(E4)
    rcb_np = r_c.astype(BF16)

    bias_np = (aa.sum() * (proj_w @ np.asarray(conv_b, np.float32))
               + np.asarray(proj_b, np.float32)).astype(np.float32)
    return (mth_np, mtl_np, ra_np, rab_np, rc_np, rcb_np, bias_np, 1.0 / sr)


def _window_x(xpad):
    """[B, D, XCOLS] (fp8) -> [B, 128, NJ, NQ, 2, 2, 128] windowed layout."""
    # value[b, p, J, q, tap, slot, m] = xpad[b, (2q+slot)*128 + p, 128J+tap+m]
    dt = xpad.dtype
    xc = xpad.view(np.uint8).reshape(B, NQ, 2, 128, XCOLS)  # [b, q, slot, p, col]
    win = np.lib.stride_tricks.sliding_window_view(
        xc, 128, axis=-1)                              # [b, q, slot, p, c0, m]
    c0 = (128 * np.arange(NJ)[:, None] + np.arange(2)[None, :])  # [J, tap]
    w = win[:, :, :, :, c0]                            # [b, q, slot, p, J, tap, m]
    return np.ascontiguousarray(w.transpose(0, 3, 4, 1, 5, 2, 6)).view(dt)


def kernel(x, conv_w, conv_b, aa_kernel, proj_w, proj_b):
    (mth_np, mtl_np, ra_np, rab_np, rc_np, rcb_np, bias_np,
     srinv) = _host_weights(conv_w, conv_b, aa_kernel, proj_w, proj_b)
    if "nc" not in _CACHE:
        _CACHE["nc"] = _build_bass(srinv)
    nc = _CACHE["nc"]

    x = np.asarray(x, np.float32)
    xh = np.zeros((B, D, XCOLS), E4)
    xl = np.zeros((B, D, XCOLS), E5)
    xh[:, :, :L] = x.astype(E4)
    xl[:, :, :L] = (x - xh[:, :, :L].astype(np.float32)).astype(E5)
    xh_w = _window_x(xh)
    xl_w = _window_x(xl)
    in_maps = [
        {"xh": xh_w[b], "xl": xl_w[b], "mth": mth_np, "mtl": mtl_np,
         "ra": ra_np, "rab": rab_np, "rc": rc_np, "rcb": rcb_np,
         "bias": bias_np}
        for b in range(B)
    ]
    try:
        res = run_bass_kernel_spmd(nc, in_maps, core_ids=list(range(B)))
    except ModuleNotFoundError:
        # axon tunnel without NTFF profiling hooks + BASS_TRACE set in the
        # environment: retry untraced
        import os
        os.environ["BASS_NEVER_TRACE"] = "1"
        res = run_bass_kernel_spmd(nc, in_maps, core_ids=list(range(B)))
    _CACHE["last_results"] = res
    return np.stack([r["out"].astype(np.float32) for r in res.results], axis=0)
